# revision 4
# baseline (speedup 1.0000x reference)
"""Trainium2 Bass kernel for nn_ChainOfThought (fusion MLP + 3-step decoder).

Strategy: pure data parallelism over batch (B=2048 -> 256 rows/core on 8
cores). All GEMMs run on the PE array as fp32r (full-rate fp32) with the
contraction dim on partitions. Weights arrive in natural [N, K] layout and are
transposed on-the-fly by the tensor engine (128x128 transposes packed 4-per-
PSUM-bank, cast-copied to SBUF as float32r, which also satisfies the
"rounded to FP32r" producer rule). The fusion block streams F=16384 in 128-row
tiles, accumulating z = (h1*h2) @ fc3_w.T into 4 persistent PSUM banks across
all 128 tiles. The decoder keeps activations batch-major [128b x free] for
layernorm/attention (free-dim reductions) and re-transposes activations
(16 PE transposes) whenever they feed a GEMM contraction.
"""

import sys

if "/opt/trn_rl_repo" not in sys.path:
    sys.path.insert(0, "/opt/trn_rl_repo")

import numpy as np

import concourse.bass as bass
import concourse.tile as tile
from concourse import bacc, mybir
from concourse.bass_utils import run_bass_kernel_spmd
from concourse.masks import make_identity

F32 = mybir.dt.float32
F32R = mybir.dt.float32r
AF = mybir.ActivationFunctionType
ALU = mybir.AluOpType
AX = mybir.AxisListType

B, DIN, E, FACTOR, STEPS, NH = 2048, 1024, 1024, 16, 3, 8
DH = E // NH
F = E * FACTOR
EPS = 1e-5
NCORES = 8
BC = B // NCORES          # 256 rows per core
NBS = BC // 128           # 2 batch subtiles
ED = E // 128             # 8 contraction chunks

_INPUT_SPECS = [
    ("image_feat", (BC, DIN)), ("text_feat", (BC, DIN)),
    ("fc1_w", (F, DIN)), ("fc1_b", (F,)),
    ("fc2_w", (F, DIN)), ("fc2_b", (F,)),
    ("fc3_w", (E, F)), ("fc3_b", (E,)),
    ("ln_g", (E,)), ("ln_b", (E,)),
    ("Wqkv", (STEPS, 3 * E, E)), ("bqkv", (STEPS, 3 * E)),
    ("Wo", (STEPS, E, E)), ("bo", (STEPS, E)),
    ("W1", (STEPS, E, E)), ("b1", (STEPS, E)),
    ("W2", (STEPS, E, E)), ("b2", (STEPS, E)),
    ("n1_g", (STEPS, E)), ("n1_b", (STEPS, E)),
    ("n2_g", (STEPS, E)), ("n2_b", (STEPS, E)),
    ("Wout", (DIN, E)), ("bout", (DIN,)),
]


def _bcast_ap(vec_ap, parts=128):
    """AP that replicates a 1-D DRAM vector across `parts` partitions."""
    return bass.AP(tensor=vec_ap.tensor, offset=vec_ap.offset,
                   ap=[[0, parts], *vec_ap.ap])


class _Builder:
    def __init__(self, nc, tc):
        self.nc = nc
        self.tc = tc
        self._cpy_i = 0
        self._bias_i = 0

    def cast_copy(self, out_ap, in_ap):
        """PSUM->SBUF copy, alternating DVE/Act engines."""
        nc = self.nc
        self._cpy_i += 1
        if self._cpy_i % 2 == 0:
            nc.vector.tensor_copy(out=out_ap, in_=in_ap)
        else:
            nc.scalar.copy(out_ap, in_ap)

    def transpose_to_T(self, ps_tr, ident, src_bm, dst_T, n_ds=ED, n_bs=NBS):
        """src_bm [128, n_bs, n_ds*128] fp32 -> dst_T [128, n_ds, n_bs*128]
        written as float32r (PE transpose + cast copy)."""
        nc = self.nc
        for bs in range(n_bs):
            for dq in range(n_ds // 4):
                pt = ps_tr.tile([128, 4, 128], F32, tag="trq")
                for j in range(4):
                    ds = dq * 4 + j
                    nc.tensor.transpose(
                        pt[:, j], src_bm[:, bs, ds * 128:(ds + 1) * 128], ident)
                self.cast_copy(
                    dst_T[:, dq * 4:(dq + 1) * 4,
                          bs * 128:(bs + 1) * 128].bitcast(F32R),
                    pt[:],
                )

    def load_wT(self, ps_tr, wnat, ident, w_rows, wT, nf):
        """w_rows: DRAM AP [nf, 1024] natural rows. wT: [128, ED, nf] f32r."""
        nc = self.nc
        for ns in range(nf // 128):
            wn = wnat.tile([128, DIN], F32, tag="wn")
            nc.sync.dma_start(wn[:], w_rows[ns * 128:(ns + 1) * 128, :])
            for dq in range(ED // 4):
                pt = ps_tr.tile([128, 4, 128], F32, tag="trq")
                for j in range(4):
                    ds = dq * 4 + j
                    nc.tensor.transpose(
                        pt[:, j], wn[:, ds * 128:(ds + 1) * 128], ident)
                self.cast_copy(
                    wT[:, dq * 4:(dq + 1) * 4,
                       ns * 128:(ns + 1) * 128].bitcast(F32R),
                    pt[:],
                )

    def mm_acc(self, psum, lhsT, rhs, n_ds=ED):
        """psum [M,N] += sum_ds lhsT[:, ds, :].T @ rhs[:, ds, :] (f32r)."""
        nc = self.nc
        for ds in range(n_ds):
            nc.tensor.matmul(
                psum,
                lhsT=lhsT[:, ds, :].bitcast(F32R),
                rhs=rhs[:, ds, :].bitcast(F32R),
                start=(ds == 0),
                stop=(ds == n_ds - 1),
            )

    def bias_from_psum(self, out_sb, psum, bias_b):
        """out_sb = psum + bias_b, alternating DVE-direct vs Act-copy+Pool-add."""
        nc = self.nc
        self._bias_i += 1
        if self._bias_i % 2 == 0:
            nc.vector.tensor_tensor(out_sb, psum, bias_b, ALU.add)
        else:
            nc.scalar.copy(out_sb, psum)
            nc.gpsimd.tensor_tensor(out_sb, out_sb, bias_b, ALU.add)

    def layernorm(self, small, x_sb, g_b, b_b, out_sb, eps_tile):
        """out = (x - mean)/sqrt(var + eps) * g + b over free dim (1024)."""
        nc = self.nc
        st = small.tile([128, 2, 6], F32, tag="bnst")
        xv = x_sb.rearrange("p (t f) -> p t f", t=2)
        nc.vector.bn_stats(st[:, 0], xv[:, 0])
        nc.vector.bn_stats(st[:, 1], xv[:, 1])
        mv = small.tile([128, 2], F32, tag="bnmv")
        nc.vector.bn_aggr(mv[:], st[:])
        rstd = small.tile([128, 1], F32, tag="rstd")
        nc.scalar.activation(rstd[:], mv[:, 1:2], AF.Sqrt, bias=eps_tile[:])
        nc.vector.reciprocal(rstd[:], rstd[:])
        nc.vector.tensor_scalar(
            out_sb, x_sb, mv[:, 0:1], rstd[:], op0=ALU.subtract, op1=ALU.mult)
        nc.gpsimd.tensor_tensor(out_sb, out_sb, g_b, ALU.mult)
        nc.gpsimd.tensor_tensor(out_sb, out_sb, b_b, ALU.add)


def build():
    nc = bacc.Bacc("TRN2", target_bir_lowering=False, debug=False)
    d = {}
    for name, shape in _INPUT_SPECS:
        d[name] = nc.dram_tensor(name, list(shape), F32, kind="ExternalInput").ap()
    final_d = nc.dram_tensor("final_out", [BC, DIN], F32, kind="ExternalOutput").ap()
    ctx_d = nc.dram_tensor("ctx_out", [BC, STEPS + 1, E], F32, kind="ExternalOutput").ap()

    with tile.TileContext(nc) as tc:
        bld = _Builder(nc, tc)
        with (
            tc.tile_pool(name="singles", bufs=1) as singles,
            tc.tile_pool(name="state", bufs=1) as state,
        ):
            ident = singles.tile([128, 128], F32, tag="ident")
            make_identity(nc, ident)
            eps_tile = singles.tile([128, 1], F32, tag="eps")
            nc.vector.memset(eps_tile, EPS)

            # persistent batch-major x and transposed ctx rows
            x_bm = state.tile([128, NBS, E], F32, tag="x_bm")
            ctxT = [state.tile([128, ED, BC], F32, name=f"ctxT{l}", tag=f"ctxT{l}")
                    for l in range(STEPS + 1)]

            # ---------------- fusion block ----------------
            with (
                tc.tile_pool(name="fus_in", bufs=1) as fus_in,
                tc.tile_pool(name="fus_w", bufs=2) as fus_w,
                tc.tile_pool(name="fus_wT", bufs=2) as fus_wT,
                tc.tile_pool(name="fus_t", bufs=2) as fus_t,
                tc.tile_pool(name="fus_small", bufs=2) as fus_small,
                tc.tile_pool(name="ps_tr", bufs=2, space="PSUM") as ps_tr,
                tc.tile_pool(name="ps_h", bufs=1, space="PSUM") as ps_h,
                tc.tile_pool(name="ps_z", bufs=1, space="PSUM") as ps_z,
            ):
                # load + transpose image/text to [d, ds, b] f32r
                img_bm = fus_in.tile([128, NBS, DIN], F32, tag="img_bm")
                nc.sync.dma_start(
                    img_bm[:], d["image_feat"].rearrange("(s p) k -> p s k", p=128))
                txt_bm = fus_in.tile([128, NBS, DIN], F32, tag="txt_bm")
                nc.sync.dma_start(
                    txt_bm[:], d["text_feat"].rearrange("(s p) k -> p s k", p=128))
                imgT = fus_in.tile([128, ED, BC], F32, tag="imgT")
                txtT = fus_in.tile([128, ED, BC], F32, tag="txtT")
                bld.transpose_to_T(ps_tr, ident, img_bm, imgT)
                bld.transpose_to_T(ps_tr, ident, txt_bm, txtT)

                # per-partition bias columns for fc1/fc2 (f on partitions)
                fc1_bS = fus_in.tile([128, F // 128], F32, tag="fc1_bS")
                nc.sync.dma_start(fc1_bS[:], d["fc1_b"].rearrange("(o p) -> p o", p=128))
                fc2_bS = fus_in.tile([128, F // 128], F32, tag="fc2_bS")
                nc.sync.dma_start(fc2_bS[:], d["fc2_b"].rearrange("(o p) -> p o", p=128))

                # z accumulators: 4 full PSUM banks [128b, 512e]
                zps = [[ps_z.tile([128, 512], F32, name=f"z{bs}{eh}", tag=f"z{bs}{eh}")
                        for eh in range(2)] for bs in range(NBS)]

                fc3_v = d["fc3_w"].rearrange("(eo p) f -> p eo f", p=128)

                for ft in range(F // 128):
                    fsl = slice(ft * 128, (ft + 1) * 128)
                    w1n = fus_w.tile([128, DIN], F32, tag="w1n")
                    nc.sync.dma_start(w1n[:], d["fc1_w"][fsl, :])
                    w2n = fus_w.tile([128, DIN], F32, tag="w2n")
                    nc.sync.dma_start(w2n[:], d["fc2_w"][fsl, :])
                    w3n = fus_w.tile([128, ED, 128], F32, tag="w3n")
                    nc.sync.dma_start(w3n[:], fc3_v[:, :, fsl])

                    # transpose fc1/fc2 tiles -> [d, ds, f128] f32r
                    w1T = fus_wT.tile([128, ED, 128], F32, tag="w1T")
                    w2T = fus_wT.tile([128, ED, 128], F32, tag="w2T")
                    w3T = fus_wT.tile([128, ED, 128], F32, tag="w3T")
                    for (wn, wT) in ((w1n, w1T), (w2n, w2T)):
                        for dq in range(2):
                            pt = ps_tr.tile([128, 4, 128], F32, tag="trq")
                            for j in range(4):
                                ds = dq * 4 + j
                                nc.tensor.transpose(
                                    pt[:, j], wn[:, ds * 128:(ds + 1) * 128], ident)
                            bld.cast_copy(
                                wT[:, dq * 4:(dq + 1) * 4, :].bitcast(F32R), pt[:])
                    # transpose fc3 tile: [e128, es, f128] -> [f128, es, e128]
                    for dq in range(2):
                        pt = ps_tr.tile([128, 4, 128], F32, tag="trq")
                        for j in range(4):
                            es = dq * 4 + j
                            nc.tensor.transpose(pt[:, j], w3n[:, es, :], ident)
                        bld.cast_copy(
                            w3T[:, dq * 4:(dq + 1) * 4, :].bitcast(F32R), pt[:])

                    h1ps = ps_h.tile([128, 512], F32, tag="h1ps")
                    h2ps = ps_h.tile([128, 512], F32, tag="h2ps")
                    bld.mm_acc(h1ps[:, :BC], w1T, imgT)
                    bld.mm_acc(h2ps[:, :BC], w2T, txtT)

                    # fused0T = (h1+b1)*(h2+b2)  [f128, 256] f32r
                    h1s = fus_small.tile([128, BC], F32, tag="h1s")
                    nc.vector.tensor_scalar(
                        h1s[:], h1ps[:, :BC], fc1_bS[:, ft:ft + 1], None, op0=ALU.add)
                    h2s = fus_small.tile([128, BC], F32, tag="h2s")
                    nc.vector.tensor_scalar(
                        h2s[:], h2ps[:, :BC], fc2_bS[:, ft:ft + 1], None, op0=ALU.add)
                    f0T = fus_small.tile([128, BC], F32, tag="f0T")
                    nc.vector.tensor_tensor(
                        f0T[:].bitcast(F32R), h1s[:], h2s[:], ALU.mult)

                    for bs in range(NBS):
                        for eh in range(2):
                            nc.tensor.matmul(
                                zps[bs][eh][:],
                                lhsT=f0T[:, bs * 128:(bs + 1) * 128].bitcast(F32R),
                                rhs=w3T[:, eh * 4:(eh + 1) * 4, :].bitcast(F32R),
                                start=(ft == 0),
                                stop=(ft == F // 128 - 1),
                            )

                # epilogue: z + fc3_b, layernorm -> x0; write ctx row 0
                fc3_bB = fus_in.tile([128, E], F32, tag="fc3_bB")
                nc.gpsimd.dma_start(fc3_bB[:], _bcast_ap(d["fc3_b"]))
                ln_gB = fus_in.tile([128, E], F32, tag="ln_gB")
                nc.gpsimd.dma_start(ln_gB[:], _bcast_ap(d["ln_g"]))
                ln_bB = fus_in.tile([128, E], F32, tag="ln_bB")
                nc.gpsimd.dma_start(ln_bB[:], _bcast_ap(d["ln_b"]))

                for bs in range(NBS):
                    z_sb = fus_t.tile([128, E], F32, tag="z_sb")
                    for eh in range(2):
                        nc.vector.tensor_tensor(
                            z_sb[:, eh * 512:(eh + 1) * 512], zps[bs][eh][:],
                            fc3_bB[:, eh * 512:(eh + 1) * 512], ALU.add)
                    bld.layernorm(fus_small, z_sb[:], ln_gB[:], ln_bB[:],
                                  x_bm[:, bs], eps_tile)
                    nc.gpsimd.dma_start(
                        ctx_d[bs * 128:(bs + 1) * 128, 0, :], x_bm[:, bs])
                bld.transpose_to_T(ps_tr, ident, x_bm, ctxT[0])

            # ---------------- decoder steps ----------------
            with (
                tc.tile_pool(name="dec_w", bufs=2) as dec_w,
                tc.tile_pool(name="dec_wT", bufs=1) as dec_wT,
                tc.tile_pool(name="dec_act", bufs=2) as dec_act,
                tc.tile_pool(name="dec_kv", bufs=2) as dec_kv,
                tc.tile_pool(name="dec_b", bufs=5) as dec_b,
                tc.tile_pool(name="dec_small", bufs=2) as dec_small,
                tc.tile_pool(name="dec_tmp", bufs=3) as dec_tmp,
                tc.tile_pool(name="ps_tr2", bufs=2, space="PSUM") as ps_tr2,
                tc.tile_pool(name="ps_mm", bufs=3, space="PSUM") as ps_mm,
            ):
                isqrt_dh = float(1.0 / np.sqrt(DH))

                def full_gemm(wT, lhsT_t, psums):
                    """psums[bs] [128,1024] = lhsT_t.T @ wT for each bs."""
                    for bs in range(NBS):
                        for eh in range(2):
                            for ds in range(ED):
                                nc.tensor.matmul(
                                    psums[bs][:, eh * 512:(eh + 1) * 512],
                                    lhsT=lhsT_t[:, ds, bs * 128:(bs + 1) * 128].bitcast(F32R),
                                    rhs=wT[:, ds, eh * 4:(eh + 1) * 4, :].bitcast(F32R),
                                    start=(ds == 0),
                                    stop=(ds == ED - 1),
                                )

                for i in range(STEPS):
                    L = i + 1
                    wT = dec_wT.tile([128, ED, ED, 128], F32, tag="wT")
                    wTv = wT.rearrange("p a b c -> p a (b c)")

                    # ---- Q = x @ Wq.T + bq ----
                    bld.load_wT(ps_tr2, dec_w, ident, d["Wqkv"][i, 0:E, :], wTv, E)
                    bqB = dec_b.tile([128, E], F32, tag="bias")
                    nc.gpsimd.dma_start(bqB[:], _bcast_ap(d["bqkv"][i, 0:E]))
                    qps = [ps_mm.tile([128, E], F32, name=f"qps{bs}", tag="mm") for bs in range(NBS)]
                    full_gemm(wT, ctxT[i], qps)
                    q_bm = dec_act.tile([128, NBS, E], F32, tag="abm")
                    for bs in range(NBS):
                        bld.bias_from_psum(q_bm[:, bs], qps[bs][:], bqB[:])

                    # ---- K rows + scores ----
                    bld.load_wT(ps_tr2, dec_w, ident, d["Wqkv"][i, E:2 * E, :], wTv, E)
                    bkB = dec_b.tile([128, E], F32, tag="bias")
                    nc.gpsimd.dma_start(bkB[:], _bcast_ap(d["bqkv"][i, E:2 * E]))
                    esc = dec_small.tile([128, NBS, NH, L], F32, tag=f"esc{i}")
                    for l in range(L):
                        kps = [ps_mm.tile([128, E], F32, name=f"kps{bs}", tag="mm") for bs in range(NBS)]
                        full_gemm(wT, ctxT[l], kps)
                        k_bm = dec_kv.tile([128, NBS, E], F32, tag="kv")
                        for bs in range(NBS):
                            bld.bias_from_psum(k_bm[:, bs], kps[bs][:], bkB[:])
                        for bs in range(NBS):
                            tmp = dec_tmp.tile([128, E], F32, tag="tmp")
                            nc.vector.tensor_tensor(
                                tmp[:], q_bm[:, bs], k_bm[:, bs], ALU.mult)
                            nc.vector.tensor_reduce(
                                out=esc[:, bs, :, l],
                                in_=tmp.rearrange("p (h e) -> p h e", h=NH),
                                axis=AX.X, op=ALU.add)

                    # ---- softmax -> probs ----
                    nc.scalar.activation(
                        esc[:].rearrange("p s h l -> p (s h l)"),
                        esc[:].rearrange("p s h l -> p (s h l)"),
                        AF.Exp, scale=isqrt_dh)
                    den = dec_small.tile([128, NBS, NH], F32, tag=f"den{i}")
                    nc.vector.tensor_reduce(
                        out=den[:].rearrange("p s h -> p (s h)"),
                        in_=esc[:].rearrange("p s h l -> p (s h) l"),
                        axis=AX.X, op=ALU.add)
                    nc.vector.reciprocal(
                        den[:].rearrange("p s h -> p (s h)"),
                        den[:].rearrange("p s h -> p (s h)"))
                    probs = dec_small.tile([128, NBS, NH, L], F32, tag=f"pr{i}")
                    nc.vector.tensor_tensor(
                        probs[:], esc[:],
                        den[:, :, :, None].to_broadcast((128, NBS, NH, L)),
                        ALU.mult)

                    # ---- V rows + weighted accumulation ----
                    bld.load_wT(ps_tr2, dec_w, ident, d["Wqkv"][i, 2 * E:3 * E, :], wTv, E)
                    bvB = dec_b.tile([128, E], F32, tag="bias")
                    nc.gpsimd.dma_start(bvB[:], _bcast_ap(d["bqkv"][i, 2 * E:3 * E]))
                    acc = dec_act.tile([128, NBS, E], F32, tag="abm")
                    for l in range(L):
                        vps = [ps_mm.tile([128, E], F32, name=f"vps{bs}", tag="mm") for bs in range(NBS)]
                        full_gemm(wT, ctxT[l], vps)
                        v_bm = dec_kv.tile([128, NBS, E], F32, tag="kv")
                        for bs in range(NBS):
                            bld.bias_from_psum(v_bm[:, bs], vps[bs][:], bvB[:])
                        for bs in range(NBS):
                            pb = probs[:, bs, :, l:l + 1].to_broadcast((128, NH, DH))
                            accv = acc[:, bs].rearrange("p (h e) -> p h e", h=NH)
                            if l == 0:
                                nc.vector.tensor_tensor(
                                    accv, v_bm[:, bs].rearrange("p (h e) -> p h e", h=NH),
                                    pb, ALU.mult)
                            else:
                                t3 = dec_tmp.tile([128, E], F32, tag="tmp")
                                nc.vector.tensor_tensor(
                                    t3[:].rearrange("p (h e) -> p h e", h=NH),
                                    v_bm[:, bs].rearrange("p (h e) -> p h e", h=NH),
                                    pb, ALU.mult)
                                nc.gpsimd.tensor_tensor(
                                    acc[:, bs], acc[:, bs], t3[:], ALU.add)

                    # ---- O projection + LN + residual ----
                    accT = dec_act.tile([128, ED, BC], F32, tag="aT")
                    bld.transpose_to_T(ps_tr2, ident, acc, accT)
                    bld.load_wT(ps_tr2, dec_w, ident, d["Wo"][i], wTv, E)
                    boB = dec_b.tile([128, E], F32, tag="bias")
                    nc.gpsimd.dma_start(boB[:], _bcast_ap(d["bo"][i]))
                    n1gB = dec_b.tile([128, E], F32, tag="bias")
                    nc.gpsimd.dma_start(n1gB[:], _bcast_ap(d["n1_g"][i]))
                    n1bB = dec_b.tile([128, E], F32, tag="bias")
                    nc.gpsimd.dma_start(n1bB[:], _bcast_ap(d["n1_b"][i]))
                    ops = [ps_mm.tile([128, E], F32, name=f"ops{bs}", tag="mm") for bs in range(NBS)]
                    full_gemm(wT, accT, ops)
                    for bs in range(NBS):
                        o_sb = dec_tmp.tile([128, E], F32, tag="tmp")
                        bld.bias_from_psum(o_sb[:], ops[bs][:], boB[:])
                        lno = dec_tmp.tile([128, E], F32, tag="tmp")
                        bld.layernorm(dec_small, o_sb[:], n1gB[:], n1bB[:],
                                      lno[:], eps_tile)
                        nc.gpsimd.tensor_tensor(
                            x_bm[:, bs], x_bm[:, bs], lno[:], ALU.add)

                    # ---- FFN ----
                    xT1 = dec_act.tile([128, ED, BC], F32, tag="aT")
                    bld.transpose_to_T(ps_tr2, ident, x_bm, xT1)
                    bld.load_wT(ps_tr2, dec_w, ident, d["W1"][i], wTv, E)
                    b1B = dec_b.tile([128, E], F32, tag="bias")
                    nc.gpsimd.dma_start(b1B[:], _bcast_ap(d["b1"][i]))
                    hps = [ps_mm.tile([128, E], F32, name=f"hps{bs}", tag="mm") for bs in range(NBS)]
                    full_gemm(wT, xT1, hps)
                    h_bm = dec_act.tile([128, NBS, E], F32, tag="abm")
                    for bs in range(NBS):
                        bld.bias_from_psum(h_bm[:, bs], hps[bs][:], b1B[:])
                        nc.scalar.activation(h_bm[:, bs], h_bm[:, bs], AF.Relu)

                    hT = dec_act.tile([128, ED, BC], F32, tag="aT")
                    bld.transpose_to_T(ps_tr2, ident, h_bm, hT)
                    bld.load_wT(ps_tr2, dec_w, ident, d["W2"][i], wTv, E)
                    b2B = dec_b.tile([128, E], F32, tag="bias")
                    nc.gpsimd.dma_start(b2B[:], _bcast_ap(d["b2"][i]))
                    n2gB = dec_b.tile([128, E], F32, tag="bias")
                    nc.gpsimd.dma_start(n2gB[:], _bcast_ap(d["n2_g"][i]))
                    n2bB = dec_b.tile([128, E], F32, tag="bias")
                    nc.gpsimd.dma_start(n2bB[:], _bcast_ap(d["n2_b"][i]))
                    fps = [ps_mm.tile([128, E], F32, name=f"fps{bs}", tag="mm") for bs in range(NBS)]
                    full_gemm(wT, hT, fps)
                    for bs in range(NBS):
                        f_sb = dec_tmp.tile([128, E], F32, tag="tmp")
                        bld.bias_from_psum(f_sb[:], fps[bs][:], b2B[:])
                        lnf = dec_tmp.tile([128, E], F32, tag="tmp")
                        bld.layernorm(dec_small, f_sb[:], n2gB[:], n2bB[:],
                                      lnf[:], eps_tile)
                        nc.gpsimd.tensor_tensor(
                            x_bm[:, bs], x_bm[:, bs], lnf[:], ALU.add)
                        nc.gpsimd.dma_start(
                            ctx_d[bs * 128:(bs + 1) * 128, i + 1, :], x_bm[:, bs])
                    bld.transpose_to_T(ps_tr2, ident, x_bm, ctxT[i + 1])

                # ---- final projection ----
                wT = dec_wT.tile([128, ED, ED, 128], F32, tag="wT")
                wTv = wT.rearrange("p a b c -> p a (b c)")
                bld.load_wT(ps_tr2, dec_w, ident, d["Wout"], wTv, DIN)
                boutB = dec_b.tile([128, DIN], F32, tag="bias")
                nc.gpsimd.dma_start(boutB[:], _bcast_ap(d["bout"]))
                outps = [ps_mm.tile([128, DIN], F32, name=f"outps{bs}", tag="mm") for bs in range(NBS)]
                full_gemm(wT, ctxT[STEPS], outps)
                for bs in range(NBS):
                    fin = dec_tmp.tile([128, DIN], F32, tag="tmp")
                    bld.bias_from_psum(fin[:], outps[bs][:], boutB[:])
                    nc.gpsimd.dma_start(
                        final_d[bs * 128:(bs + 1) * 128, :], fin[:])

    nc.compile()
    return nc


_NC_CACHE = None


def _get_nc():
    global _NC_CACHE
    if _NC_CACHE is None:
        _NC_CACHE = build()
    return _NC_CACHE


def kernel(**inputs):
    nc = _get_nc()
    arrs = {k: np.ascontiguousarray(np.asarray(v, dtype=np.float32))
            for k, v in inputs.items()}
    in_maps = []
    for c in range(NCORES):
        m = dict(arrs)
        m["image_feat"] = arrs["image_feat"][c * BC:(c + 1) * BC]
        m["text_feat"] = arrs["text_feat"][c * BC:(c + 1) * BC]
        in_maps.append(m)
    res = run_bass_kernel_spmd(nc, in_maps, core_ids=list(range(NCORES)))
    final = np.concatenate([res.results[c]["final_out"] for c in range(NCORES)], axis=0)
    ctx = np.concatenate([res.results[c]["ctx_out"] for c in range(NCORES)], axis=0)
    return final, ctx


# revision 13
# speedup vs baseline: 49.4878x; 49.4878x over previous
"""Trainium2 Bass kernel for nn_ChainOfThought (fusion MLP + 3-step decoder).

Strategy: pure data parallelism over batch (B=2048 -> 256 rows/core on 8
cores). All GEMMs run on the PE array as fp32r (full-rate fp32) with the
contraction dim on partitions. Weights arrive in natural [N, K] layout and are
transposed on-the-fly by the tensor engine (128x128 transposes packed 4-per-
PSUM-bank, cast-copied to SBUF as float32r, which also satisfies the
"rounded to FP32r" producer rule). The fusion block streams F=16384 in 128-row
tiles, accumulating z = (h1*h2) @ fc3_w.T into 4 persistent PSUM banks across
all 128 tiles. The decoder keeps activations batch-major [128b x free] for
layernorm/attention (free-dim reductions) and re-transposes activations
(16 PE transposes) whenever they feed a GEMM contraction.
"""

import sys

if "/opt/trn_rl_repo" not in sys.path:
    sys.path.insert(0, "/opt/trn_rl_repo")

import numpy as np

import concourse.bass as bass
import concourse.tile as tile
from concourse import bacc, mybir
from concourse.bass_utils import run_bass_kernel_spmd
from concourse.masks import make_identity

F32 = mybir.dt.float32
F32R = mybir.dt.float32r
AF = mybir.ActivationFunctionType
ALU = mybir.AluOpType
AX = mybir.AxisListType

B, DIN, E, FACTOR, STEPS, NH = 2048, 1024, 1024, 16, 3, 8
DH = E // NH
F = E * FACTOR
EPS = 1e-5
NCORES = 8
BC = B // NCORES          # 256 rows per core
NBS = BC // 128           # 2 batch subtiles
ED = E // 128             # 8 contraction chunks

FL = F // NCORES          # 2048 local fusion rows per core
FLT = FL // 128           # 16 local f-tiles
HB = B // 2               # 1024-row batch half
HQ = HB // 128            # 8 chunks per half

_INPUT_SPECS = [
    ("image_feat", (B, DIN)), ("text_feat", (B, DIN)),
    ("fc1_wl", (FL, DIN)), ("fc1_bl", (FL,)),
    ("fc2_wl", (FL, DIN)), ("fc2_bl", (FL,)),
    ("fc3_wl", (E, FL)), ("fc3_b", (E,)),
    ("ln_g", (E,)), ("ln_b", (E,)),
    ("Wqkv", (STEPS, 3 * E, E)), ("bqkv", (STEPS, 3 * E)),
    ("Wo", (STEPS, E, E)), ("bo", (STEPS, E)),
    ("W1", (STEPS, E, E)), ("b1", (STEPS, E)),
    ("W2", (STEPS, E, E)), ("b2", (STEPS, E)),
    ("n1_g", (STEPS, E)), ("n1_b", (STEPS, E)),
    ("n2_g", (STEPS, E)), ("n2_b", (STEPS, E)),
    ("Wout", (DIN, E)), ("bout", (DIN,)),
]


def _bcast_ap(vec_ap, parts=128):
    """AP that replicates a 1-D DRAM vector across `parts` partitions."""
    return bass.AP(tensor=vec_ap.tensor, offset=vec_ap.offset,
                   ap=[[0, parts], *vec_ap.ap])


class _Builder:
    def __init__(self, nc, tc):
        self.nc = nc
        self.tc = tc
        self._cpy_i = 0
        self._bias_i = 0

    def cast_copy(self, out_ap, in_ap):
        """PSUM->SBUF copy, alternating DVE/Act engines."""
        nc = self.nc
        self._cpy_i += 1
        if self._cpy_i % 2 == 0:
            nc.vector.tensor_copy(out=out_ap, in_=in_ap)
        else:
            nc.scalar.copy(out_ap, in_ap)

    def transpose_to_T(self, ps_tr, ident, src_bm, dst_T, n_ds=ED, n_bs=NBS):
        """src_bm [128, n_bs, n_ds*128] fp32 -> dst_T [128, n_ds, n_bs*128]
        written as float32r (PE transpose + cast copy)."""
        nc = self.nc
        for bs in range(n_bs):
            for dq in range(n_ds // 4):
                pt = ps_tr.tile([128, 4, 128], F32, tag="trq")
                for j in range(4):
                    ds = dq * 4 + j
                    nc.tensor.transpose(
                        pt[:, j], src_bm[:, bs, ds * 128:(ds + 1) * 128], ident)
                self.cast_copy(
                    dst_T[:, dq * 4:(dq + 1) * 4,
                          bs * 128:(bs + 1) * 128].bitcast(F32R),
                    pt[:],
                )

    def load_wT(self, ps_tr, wnat, ident, w_rows, wT, nf):
        """w_rows: DRAM AP [nf, 1024] natural rows. wT: [128, ED, nf] f32r."""
        nc = self.nc
        for ns in range(nf // 128):
            wn = wnat.tile([128, DIN], F32, tag="wn")
            nc.sync.dma_start(wn[:], w_rows[ns * 128:(ns + 1) * 128, :])
            for dq in range(ED // 4):
                pt = ps_tr.tile([128, 4, 128], F32, tag="trq")
                for j in range(4):
                    ds = dq * 4 + j
                    nc.tensor.transpose(
                        pt[:, j], wn[:, ds * 128:(ds + 1) * 128], ident)
                self.cast_copy(
                    wT[:, dq * 4:(dq + 1) * 4,
                       ns * 128:(ns + 1) * 128].bitcast(F32R),
                    pt[:],
                )

    def mm_acc(self, psum, lhsT, rhs, n_ds=ED):
        """psum [M,N] += sum_ds lhsT[:, ds, :].T @ rhs[:, ds, :] (f32r)."""
        nc = self.nc
        for ds in range(n_ds):
            nc.tensor.matmul(
                psum,
                lhsT=lhsT[:, ds, :].bitcast(F32R),
                rhs=rhs[:, ds, :].bitcast(F32R),
                start=(ds == 0),
                stop=(ds == n_ds - 1),
            )

    def bias_from_psum(self, out_sb, psum, bias_b):
        """out_sb = psum + bias_b, alternating DVE-direct vs Act-copy+Pool-add."""
        nc = self.nc
        self._bias_i += 1
        if self._bias_i % 2 == 0:
            nc.vector.tensor_tensor(out_sb, psum, bias_b, ALU.add)
        else:
            nc.scalar.copy(out_sb, psum)
            nc.gpsimd.tensor_tensor(out_sb, out_sb, bias_b, ALU.add)

    def layernorm(self, small, x_sb, g_b, b_b, out_sb, eps_tile):
        """out = (x - mean)/sqrt(var + eps) * g + b over free dim (1024)."""
        nc = self.nc
        st = small.tile([128, 2, 6], F32, tag="bnst")
        xv = x_sb.rearrange("p (t f) -> p t f", t=2)
        nc.vector.bn_stats(st[:, 0], xv[:, 0])
        nc.vector.bn_stats(st[:, 1], xv[:, 1])
        mv = small.tile([128, 2], F32, tag="bnmv")
        nc.vector.bn_aggr(mv[:], st[:])
        rstd = small.tile([128, 1], F32, tag="rstd")
        nc.scalar.activation(rstd[:], mv[:, 1:2], AF.Sqrt, bias=eps_tile[:])
        nc.vector.reciprocal(rstd[:], rstd[:])
        nc.vector.tensor_scalar(
            out_sb, x_sb, mv[:, 0:1], rstd[:], op0=ALU.subtract, op1=ALU.mult)
        nc.gpsimd.tensor_tensor(out_sb, out_sb, g_b, ALU.mult)
        nc.gpsimd.tensor_tensor(out_sb, out_sb, b_b, ALU.add)


def build(reps=1, parts="all"):
    nc = bacc.Bacc("TRN2", target_bir_lowering=False, debug=False)
    d = {}
    for name, shape in _INPUT_SPECS:
        d[name] = nc.dram_tensor(name, list(shape), F32, kind="ExternalInput").ap()
    outs = []
    for r in range(reps):
        sfx = "" if r == 0 else f"_{r}"
        outs.append((
            nc.dram_tensor(f"final_out{sfx}", [BC, DIN], F32, kind="ExternalOutput").ap(),
            nc.dram_tensor(f"ctx_out{sfx}", [BC, STEPS + 1, E], F32, kind="ExternalOutput").ap(),
        ))

    with tile.TileContext(nc) as tc:
        bld = _Builder(nc, tc)
        for final_d, ctx_d in outs:
            _emit_body(nc, tc, bld, d, final_d, ctx_d, parts)

    nc.compile()
    return nc


def _emit_body(nc, tc, bld, d, final_d, ctx_d, parts="all"):
    if True:
        with (
            tc.tile_pool(name="singles", bufs=1) as singles,
            tc.tile_pool(name="state", bufs=1) as state,
        ):
            ident = singles.tile([128, 128], F32, tag="ident")
            make_identity(nc, ident)
            eps_tile = singles.tile([128, 1], F32, tag="eps")
            nc.vector.memset(eps_tile, EPS)

            # persistent batch-major x (ctx rows live in the decoder scope)
            x_bm = state.tile([128, NBS, E], F32, tag="x_bm")

            # ---------------- fusion block (F-sharded + ReduceScatter) ----
            if parts in ("all", "fusion"):
              rs_in = nc.dram_tensor(f"rs_in_{nc.next_id()}", [2, HB, E], F32).ap()
              rs_out = nc.dram_tensor(f"rs_out_{nc.next_id()}", [2, HB // NCORES, E], F32).ap()
              with (
                tc.tile_pool(name="fus_in", bufs=2) as fus_in,
                tc.tile_pool(name="fus_T", bufs=1) as fus_T,
                tc.tile_pool(name="fus_w", bufs=2) as fus_w,
                tc.tile_pool(name="fus_wT", bufs=1) as fus_wT,
                tc.tile_pool(name="fus_t", bufs=1) as fus_t,
                tc.tile_pool(name="fus_small", bufs=1) as fus_small,
                tc.tile_pool(name="ps_tr", bufs=2, space="PSUM") as ps_tr,
                tc.tile_pool(name="ps_h", bufs=1, space="PSUM") as ps_h,
                tc.tile_pool(name="ps_zc", bufs=1, space="PSUM") as ps_zc,
              ):
                fc1_bSl = fus_T.tile([128, FLT], F32, tag="fc1_bSl")
                nc.sync.dma_start(fc1_bSl[:], d["fc1_bl"].rearrange("(o p) -> p o", p=128))
                fc2_bSl = fus_T.tile([128, FLT], F32, tag="fc2_bSl")
                nc.sync.dma_start(fc2_bSl[:], d["fc2_bl"].rearrange("(o p) -> p o", p=128))
                fc3_v = d["fc3_wl"].rearrange("(eo p) f -> p eo f", p=128)

                for hp in range(2):  # batch-half pass
                    imgT = fus_T.tile([128, ED, HB], F32, tag="imgT")
                    txtT = fus_T.tile([128, ED, HB], F32, tag="txtT")
                    for (src, dstT) in ((d["image_feat"], imgT), (d["text_feat"], txtT)):
                        for bq in range(HQ):
                            chunk = fus_in.tile([128, DIN], F32, tag="chunk")
                            nc.sync.dma_start(
                                chunk[:],
                                src[hp * HB + bq * 128:hp * HB + (bq + 1) * 128, :])
                            for dq in range(2):
                                pt = ps_tr.tile([128, 4, 128], F32, tag="trq")
                                for j in range(4):
                                    ds = dq * 4 + j
                                    nc.tensor.transpose(
                                        pt[:, j], chunk[:, ds * 128:(ds + 1) * 128], ident)
                                bld.cast_copy(
                                    dstT[:, dq * 4:(dq + 1) * 4,
                                         bq * 128:(bq + 1) * 128].bitcast(F32R),
                                    pt[:])

                    z_sb = fus_T.tile([128, HQ, E], F32, tag="z_sb")
                    for ft in range(FLT):
                        fsl = slice(ft * 128, (ft + 1) * 128)
                        w1n = fus_w.tile([128, DIN], F32, tag="w1n")
                        nc.sync.dma_start(w1n[:], d["fc1_wl"][fsl, :])
                        w2n = fus_w.tile([128, DIN], F32, tag="w2n")
                        nc.sync.dma_start(w2n[:], d["fc2_wl"][fsl, :])
                        w3n = fus_w.tile([128, ED, 128], F32, tag="w3n")
                        nc.sync.dma_start(w3n[:], fc3_v[:, :, fsl])

                        w1T = fus_wT.tile([128, ED, 128], F32, tag="w1T")
                        w2T = fus_wT.tile([128, ED, 128], F32, tag="w2T")
                        w3T = fus_wT.tile([128, ED, 128], F32, tag="w3T")
                        for (wn, wT) in ((w1n, w1T), (w2n, w2T)):
                            for dq in range(2):
                                pt = ps_tr.tile([128, 4, 128], F32, tag="trq")
                                for j in range(4):
                                    ds = dq * 4 + j
                                    nc.tensor.transpose(
                                        pt[:, j], wn[:, ds * 128:(ds + 1) * 128], ident)
                                bld.cast_copy(
                                    wT[:, dq * 4:(dq + 1) * 4, :].bitcast(F32R), pt[:])
                        for dq in range(2):
                            pt = ps_tr.tile([128, 4, 128], F32, tag="trq")
                            for j in range(4):
                                es = dq * 4 + j
                                nc.tensor.transpose(pt[:, j], w3n[:, es, :], ident)
                            bld.cast_copy(
                                w3T[:, dq * 4:(dq + 1) * 4, :].bitcast(F32R), pt[:])

                        hps = [[ps_h.tile([128, 512], F32, name=f"h{i}{hh}", tag=f"h{i}{hh}")
                                for hh in range(2)] for i in range(2)]
                        for hh in range(2):
                            bld.mm_acc(hps[0][hh][:], w1T, imgT[:, :, hh * 512:(hh + 1) * 512])
                            bld.mm_acc(hps[1][hh][:], w2T, txtT[:, :, hh * 512:(hh + 1) * 512])
                        h1s = fus_small.tile([128, HB], F32, tag="h1s")
                        h2s = fus_small.tile([128, HB], F32, tag="h2s")
                        for hh in range(2):
                            nc.scalar.activation(
                                h1s[:, hh * 512:(hh + 1) * 512], hps[0][hh][:],
                                AF.Identity, bias=fc1_bSl[:, ft:ft + 1])
                            nc.scalar.activation(
                                h2s[:, hh * 512:(hh + 1) * 512], hps[1][hh][:],
                                AF.Identity, bias=fc2_bSl[:, ft:ft + 1])
                        f0T = fus_small.tile([128, HB], F32, tag="f0T", bufs=2)
                        nc.vector.tensor_tensor(
                            f0T[:].bitcast(F32R), h1s[:], h2s[:], ALU.mult)

                        for bq in range(HQ):
                            zc = ps_zc.tile([128, E], F32, tag="zc")
                            for eh in range(2):
                                nc.tensor.matmul(
                                    zc[:, eh * 512:(eh + 1) * 512],
                                    lhsT=f0T[:, bq * 128:(bq + 1) * 128].bitcast(F32R),
                                    rhs=w3T[:, eh * 4:(eh + 1) * 4, :].bitcast(F32R),
                                    start=True, stop=True)
                            if ft == 0:
                                if bq % 2 == 0:
                                    nc.vector.tensor_copy(out=z_sb[:, bq], in_=zc[:])
                                else:
                                    nc.scalar.copy(z_sb[:, bq], zc[:])
                            elif bq % 2 == 0:
                                nc.vector.tensor_tensor(
                                    z_sb[:, bq], z_sb[:, bq], zc[:], ALU.add)
                            else:
                                t4 = fus_small.tile([128, E], F32, tag="zcp")
                                nc.scalar.copy(t4[:], zc[:])
                                nc.gpsimd.tensor_tensor(
                                    z_sb[:, bq], z_sb[:, bq], t4[:], ALU.add)

                    nc.sync.dma_start(
                        rs_in[hp].rearrange("(q pp) e -> pp q e", pp=128), z_sb[:])
                    nc.gpsimd.collective_compute(
                        "ReduceScatter", ALU.add,
                        replica_groups=[list(range(NCORES))],
                        ins=[rs_in[hp]], outs=[rs_out[hp]])

                # epilogue: zr + fc3_b -> layernorm -> x0 strips
                fc3_bB = fus_t.tile([128, E], F32, tag="fc3_bB")
                nc.gpsimd.dma_start(fc3_bB[:], _bcast_ap(d["fc3_b"]))
                ln_gB = fus_t.tile([128, E], F32, tag="ln_gB")
                nc.gpsimd.dma_start(ln_gB[:], _bcast_ap(d["ln_g"]))
                ln_bB = fus_t.tile([128, E], F32, tag="ln_bB")
                nc.gpsimd.dma_start(ln_bB[:], _bcast_ap(d["ln_b"]))
                for st in range(2):
                    zr = fus_t.tile([128, E], F32, tag="zr")
                    nc.sync.dma_start(zr[:], rs_out[st])
                    z2 = fus_t.tile([128, E], F32, tag="z2")
                    nc.vector.tensor_tensor(z2[:], zr[:], fc3_bB[:], ALU.add)
                    bld.layernorm(fus_small, z2[:], ln_gB[:], ln_bB[:],
                                  x_bm[:, st], eps_tile)
                    nc.gpsimd.dma_start(
                        ctx_d[st * 128:(st + 1) * 128, 0, :], x_bm[:, st])

            # ---------------- decoder steps ----------------
            if parts in ("all", "decoder"):
              with (
                tc.tile_pool(name="dec_w", bufs=2) as dec_w,
                tc.tile_pool(name="dec_wT", bufs=1) as dec_wT,
                tc.tile_pool(name="dec_act", bufs=2) as dec_act,
                tc.tile_pool(name="dec_kv", bufs=2) as dec_kv,
                tc.tile_pool(name="dec_b", bufs=5) as dec_b,
                tc.tile_pool(name="dec_small", bufs=2) as dec_small,
                tc.tile_pool(name="dec_tmp", bufs=3) as dec_tmp,
                tc.tile_pool(name="ps_tr2", bufs=2, space="PSUM") as ps_tr2,
                tc.tile_pool(name="ps_mm", bufs=3, space="PSUM") as ps_mm,
            ):
                isqrt_dh = float(1.0 / np.sqrt(DH))
                ctxT = [dec_wT.tile([128, ED, BC], F32, name=f"ctxT{l}", tag=f"ctxT{l}")
                        for l in range(STEPS + 1)]
                if parts == "decoder":
                    nc.sync.dma_start(
                        x_bm[:], d["image_feat"][:BC].rearrange("(s p) k -> p s k", p=128))
                    for l in range(STEPS + 1):
                        bld.transpose_to_T(ps_tr2, ident, x_bm, ctxT[l])
                else:
                    bld.transpose_to_T(ps_tr2, ident, x_bm, ctxT[0])

                def full_gemm(wT, lhsT_t, psums):
                    """psums[bs] [128,1024] = lhsT_t.T @ wT for each bs."""
                    for bs in range(NBS):
                        for eh in range(2):
                            for ds in range(ED):
                                nc.tensor.matmul(
                                    psums[bs][:, eh * 512:(eh + 1) * 512],
                                    lhsT=lhsT_t[:, ds, bs * 128:(bs + 1) * 128].bitcast(F32R),
                                    rhs=wT[:, ds, eh * 4:(eh + 1) * 4, :].bitcast(F32R),
                                    start=(ds == 0),
                                    stop=(ds == ED - 1),
                                )

                for i in range(STEPS):
                    L = i + 1
                    wT = dec_wT.tile([128, ED, ED, 128], F32, tag="wT")
                    wTv = wT.rearrange("p a b c -> p a (b c)")

                    # ---- Q = x @ Wq.T + bq ----
                    bld.load_wT(ps_tr2, dec_w, ident, d["Wqkv"][i, 0:E, :], wTv, E)
                    bqB = dec_b.tile([128, E], F32, tag="bias")
                    nc.gpsimd.dma_start(bqB[:], _bcast_ap(d["bqkv"][i, 0:E]))
                    qps = [ps_mm.tile([128, E], F32, name=f"qps{bs}", tag="mm") for bs in range(NBS)]
                    full_gemm(wT, ctxT[i], qps)
                    q_bm = dec_act.tile([128, NBS, E], F32, tag="abm")
                    for bs in range(NBS):
                        bld.bias_from_psum(q_bm[:, bs], qps[bs][:], bqB[:])

                    # ---- K rows + scores ----
                    bld.load_wT(ps_tr2, dec_w, ident, d["Wqkv"][i, E:2 * E, :], wTv, E)
                    bkB = dec_b.tile([128, E], F32, tag="bias")
                    nc.gpsimd.dma_start(bkB[:], _bcast_ap(d["bqkv"][i, E:2 * E]))
                    esc = dec_small.tile([128, NBS, NH, L], F32, tag=f"esc{i}")
                    for l in range(L):
                        kps = [ps_mm.tile([128, E], F32, name=f"kps{bs}", tag="mm") for bs in range(NBS)]
                        full_gemm(wT, ctxT[l], kps)
                        k_bm = dec_kv.tile([128, NBS, E], F32, tag="kv")
                        for bs in range(NBS):
                            bld.bias_from_psum(k_bm[:, bs], kps[bs][:], bkB[:])
                        for bs in range(NBS):
                            tmp = dec_tmp.tile([128, E], F32, tag="tmp")
                            nc.vector.tensor_tensor(
                                tmp[:], q_bm[:, bs], k_bm[:, bs], ALU.mult)
                            nc.vector.tensor_reduce(
                                out=esc[:, bs, :, l],
                                in_=tmp.rearrange("p (h e) -> p h e", h=NH),
                                axis=AX.X, op=ALU.add)

                    # ---- softmax -> probs ----
                    nc.scalar.activation(
                        esc[:].rearrange("p s h l -> p (s h l)"),
                        esc[:].rearrange("p s h l -> p (s h l)"),
                        AF.Exp, scale=isqrt_dh)
                    den = dec_small.tile([128, NBS, NH], F32, tag=f"den{i}")
                    nc.vector.tensor_reduce(
                        out=den[:].rearrange("p s h -> p (s h)"),
                        in_=esc[:].rearrange("p s h l -> p (s h) l"),
                        axis=AX.X, op=ALU.add)
                    nc.vector.reciprocal(
                        den[:].rearrange("p s h -> p (s h)"),
                        den[:].rearrange("p s h -> p (s h)"))
                    probs = dec_small.tile([128, NBS, NH, L], F32, tag=f"pr{i}")
                    nc.vector.tensor_tensor(
                        probs[:], esc[:],
                        den[:, :, :, None].to_broadcast((128, NBS, NH, L)),
                        ALU.mult)

                    # ---- V rows + weighted accumulation ----
                    bld.load_wT(ps_tr2, dec_w, ident, d["Wqkv"][i, 2 * E:3 * E, :], wTv, E)
                    bvB = dec_b.tile([128, E], F32, tag="bias")
                    nc.gpsimd.dma_start(bvB[:], _bcast_ap(d["bqkv"][i, 2 * E:3 * E]))
                    acc = dec_act.tile([128, NBS, E], F32, tag="abm")
                    for l in range(L):
                        vps = [ps_mm.tile([128, E], F32, name=f"vps{bs}", tag="mm") for bs in range(NBS)]
                        full_gemm(wT, ctxT[l], vps)
                        v_bm = dec_kv.tile([128, NBS, E], F32, tag="kv")
                        for bs in range(NBS):
                            bld.bias_from_psum(v_bm[:, bs], vps[bs][:], bvB[:])
                        for bs in range(NBS):
                            pb = probs[:, bs, :, l:l + 1].to_broadcast((128, NH, DH))
                            accv = acc[:, bs].rearrange("p (h e) -> p h e", h=NH)
                            if l == 0:
                                nc.vector.tensor_tensor(
                                    accv, v_bm[:, bs].rearrange("p (h e) -> p h e", h=NH),
                                    pb, ALU.mult)
                            else:
                                t3 = dec_tmp.tile([128, E], F32, tag="tmp")
                                nc.vector.tensor_tensor(
                                    t3[:].rearrange("p (h e) -> p h e", h=NH),
                                    v_bm[:, bs].rearrange("p (h e) -> p h e", h=NH),
                                    pb, ALU.mult)
                                nc.gpsimd.tensor_tensor(
                                    acc[:, bs], acc[:, bs], t3[:], ALU.add)

                    # ---- O projection + LN + residual ----
                    accT = dec_act.tile([128, ED, BC], F32, tag="aT")
                    bld.transpose_to_T(ps_tr2, ident, acc, accT)
                    bld.load_wT(ps_tr2, dec_w, ident, d["Wo"][i], wTv, E)
                    boB = dec_b.tile([128, E], F32, tag="bias")
                    nc.gpsimd.dma_start(boB[:], _bcast_ap(d["bo"][i]))
                    n1gB = dec_b.tile([128, E], F32, tag="bias")
                    nc.gpsimd.dma_start(n1gB[:], _bcast_ap(d["n1_g"][i]))
                    n1bB = dec_b.tile([128, E], F32, tag="bias")
                    nc.gpsimd.dma_start(n1bB[:], _bcast_ap(d["n1_b"][i]))
                    ops = [ps_mm.tile([128, E], F32, name=f"ops{bs}", tag="mm") for bs in range(NBS)]
                    full_gemm(wT, accT, ops)
                    for bs in range(NBS):
                        o_sb = dec_tmp.tile([128, E], F32, tag="tmp")
                        bld.bias_from_psum(o_sb[:], ops[bs][:], boB[:])
                        lno = dec_tmp.tile([128, E], F32, tag="tmp")
                        bld.layernorm(dec_small, o_sb[:], n1gB[:], n1bB[:],
                                      lno[:], eps_tile)
                        nc.gpsimd.tensor_tensor(
                            x_bm[:, bs], x_bm[:, bs], lno[:], ALU.add)

                    # ---- FFN ----
                    xT1 = dec_act.tile([128, ED, BC], F32, tag="aT")
                    bld.transpose_to_T(ps_tr2, ident, x_bm, xT1)
                    bld.load_wT(ps_tr2, dec_w, ident, d["W1"][i], wTv, E)
                    b1B = dec_b.tile([128, E], F32, tag="bias")
                    nc.gpsimd.dma_start(b1B[:], _bcast_ap(d["b1"][i]))
                    hps = [ps_mm.tile([128, E], F32, name=f"hps{bs}", tag="mm") for bs in range(NBS)]
                    full_gemm(wT, xT1, hps)
                    h_bm = dec_act.tile([128, NBS, E], F32, tag="abm")
                    for bs in range(NBS):
                        bld.bias_from_psum(h_bm[:, bs], hps[bs][:], b1B[:])
                        nc.scalar.activation(h_bm[:, bs], h_bm[:, bs], AF.Relu)

                    hT = dec_act.tile([128, ED, BC], F32, tag="aT")
                    bld.transpose_to_T(ps_tr2, ident, h_bm, hT)
                    bld.load_wT(ps_tr2, dec_w, ident, d["W2"][i], wTv, E)
                    b2B = dec_b.tile([128, E], F32, tag="bias")
                    nc.gpsimd.dma_start(b2B[:], _bcast_ap(d["b2"][i]))
                    n2gB = dec_b.tile([128, E], F32, tag="bias")
                    nc.gpsimd.dma_start(n2gB[:], _bcast_ap(d["n2_g"][i]))
                    n2bB = dec_b.tile([128, E], F32, tag="bias")
                    nc.gpsimd.dma_start(n2bB[:], _bcast_ap(d["n2_b"][i]))
                    fps = [ps_mm.tile([128, E], F32, name=f"fps{bs}", tag="mm") for bs in range(NBS)]
                    full_gemm(wT, hT, fps)
                    for bs in range(NBS):
                        f_sb = dec_tmp.tile([128, E], F32, tag="tmp")
                        bld.bias_from_psum(f_sb[:], fps[bs][:], b2B[:])
                        lnf = dec_tmp.tile([128, E], F32, tag="tmp")
                        bld.layernorm(dec_small, f_sb[:], n2gB[:], n2bB[:],
                                      lnf[:], eps_tile)
                        nc.gpsimd.tensor_tensor(
                            x_bm[:, bs], x_bm[:, bs], lnf[:], ALU.add)
                        nc.gpsimd.dma_start(
                            ctx_d[bs * 128:(bs + 1) * 128, i + 1, :], x_bm[:, bs])
                    bld.transpose_to_T(ps_tr2, ident, x_bm, ctxT[i + 1])

                # ---- final projection ----
                wT = dec_wT.tile([128, ED, ED, 128], F32, tag="wT")
                wTv = wT.rearrange("p a b c -> p a (b c)")
                bld.load_wT(ps_tr2, dec_w, ident, d["Wout"], wTv, DIN)
                boutB = dec_b.tile([128, DIN], F32, tag="bias")
                nc.gpsimd.dma_start(boutB[:], _bcast_ap(d["bout"]))
                outps = [ps_mm.tile([128, DIN], F32, name=f"outps{bs}", tag="mm") for bs in range(NBS)]
                full_gemm(wT, ctxT[STEPS], outps)
                for bs in range(NBS):
                    fin = dec_tmp.tile([128, DIN], F32, tag="tmp")
                    bld.bias_from_psum(fin[:], outps[bs][:], boutB[:])
                    nc.gpsimd.dma_start(
                        final_d[bs * 128:(bs + 1) * 128, :], fin[:])


_NC_CACHE = None


def _get_nc():
    global _NC_CACHE
    if _NC_CACHE is None:
        _NC_CACHE = build()
    return _NC_CACHE


def make_in_maps(arrs):
    in_maps = []
    for c in range(NCORES):
        m = {k: v for k, v in arrs.items()
             if k not in ("fc1_w", "fc1_b", "fc2_w", "fc2_b", "fc3_w")}
        m["fc1_wl"] = arrs["fc1_w"][c * FL:(c + 1) * FL]
        m["fc1_bl"] = arrs["fc1_b"][c * FL:(c + 1) * FL]
        m["fc2_wl"] = arrs["fc2_w"][c * FL:(c + 1) * FL]
        m["fc2_bl"] = arrs["fc2_b"][c * FL:(c + 1) * FL]
        m["fc3_wl"] = np.ascontiguousarray(arrs["fc3_w"][:, c * FL:(c + 1) * FL])
        in_maps.append(m)
    return in_maps


def kernel(**inputs):
    nc = _get_nc()
    arrs = {k: np.ascontiguousarray(np.asarray(v, dtype=np.float32))
            for k, v in inputs.items()}
    in_maps = make_in_maps(arrs)
    res = run_bass_kernel_spmd(nc, in_maps, core_ids=list(range(NCORES)))
    # core c owns batch strips {c*128..} of each half
    final = np.empty((B, DIN), np.float32)
    ctx = np.empty((B, STEPS + 1, E), np.float32)
    for c in range(NCORES):
        rf = res.results[c]["final_out"]
        rc = res.results[c]["ctx_out"]
        for st in range(2):
            g0 = st * HB + c * 128
            final[g0:g0 + 128] = rf[st * 128:(st + 1) * 128]
            ctx[g0:g0 + 128] = rc[st * 128:(st + 1) * 128]
    return final, ctx
